# revision 1
# baseline (speedup 1.0000x reference)
"""TRN2 Bass kernel for nn_EdgeMLP: masked pairwise cosine similarity.

out[i, j] = [cls1_i == cls2_j] * cos(f(e1_i), f(e2_j)),  f = 2-layer MLP.

Strategy (8 cores, data-parallel over edges1 rows):
  - Host: sort edges2 columns by class label (pure data movement), so the
    class-equality mask becomes contiguous column segments.  Each core gets
    a 1024-row shard of edges1 and the full sorted edges2.
  - Device: fully pipelined over 1024-col output chunks.  Per chunk: MLP
    (fp32 matmuls), column norms via a ones-matmul (sums replicated across
    32 partitions), fused bias+normalize straight from PSUM, bf16 hi/lo
    split, then one matmul per (class segment x 128-row tile), each split
    on the 512-col psum-bank grid.  Masked entries are exact zeros (the
    class-gated lhsT column is all-zero).  The edges1-side prep (a long
    serial chain) is emitted interleaved into the first chunks so the
    static per-engine schedule keeps all engines busy; main matmuls lag
    the prologue stream by LAG chunks.
  - Host: concatenate row shards, scatter columns back to original order.

MODE selects main-matmul precision:
  "f32"   exact fp32 (4 cyc/row)
  "f32r"  tf32-like fast mode (1 cyc/row, ~1.5e-4 rel err)
  "split" bf16 hi/lo 3-term split packed into one K=96 matmul
          (1 cyc/row, ~1e-5 rel err)
"""

import sys

for _p in ("/opt/trn_rl_repo", "/opt/pypackages"):
    if _p not in sys.path:
        sys.path.append(_p)

from contextlib import ExitStack

import ml_dtypes
import numpy as np

import concourse.bass as bass
import concourse.tile as tile
from concourse import bacc, mybir
from concourse.bass_utils import run_bass_kernel_spmd

F32 = mybir.dt.float32
F32R = mybir.dt.float32r
BF16 = mybir.dt.bfloat16
AF = mybir.ActivationFunctionType
ALU = mybir.AluOpType

N1, N2 = 8192, 8192
NCORES = 8
MLOC = N1 // NCORES  # 1024
DH, DF, NCLS = 64, 32, 8
CH = 512  # psum-bank / fp32-moving-max grid

MODE = "split"

_cache: dict = {}


def _build_program(counts: tuple, mode: str, reps: int = 1):
    """Build the per-core Bacc program. `counts` = class histogram of the
    (sorted) edges2 columns; segment boundaries are baked into the loop
    structure. `reps` repeats the whole body (timing use only)."""
    bounds = np.concatenate([[0], np.cumsum(counts)]).astype(int)

    nc = bacc.Bacc("TRN2", target_bir_lowering=False, debug=False)

    e2t_d = nc.dram_tensor("e2t", [3, N2], F32, kind="ExternalInput").ap()
    e1t_d = nc.dram_tensor("e1t", [3, MLOC], F32, kind="ExternalInput").ap()
    cls1_d = nc.dram_tensor("cls1", [DF, MLOC], BF16, kind="ExternalInput").ap()
    w1_d = nc.dram_tensor("w1", [3, DH], F32, kind="ExternalInput").ap()
    b1_d = nc.dram_tensor("b1", [DH, 1], F32, kind="ExternalInput").ap()
    w2_d = nc.dram_tensor("w2", [DH, DF], F32, kind="ExternalInput").ap()
    b2_d = nc.dram_tensor("b2", [DF, 1], F32, kind="ExternalInput").ap()
    ones_d = nc.dram_tensor("ones", [DF, DF], F32, kind="ExternalInput").ap()
    out_d = nc.dram_tensor("out", [MLOC, N2], F32, kind="ExternalOutput").ap()

    with tile.TileContext(nc) as tc:
        for _rep in range(reps):
            _emit_body(nc, tc, bounds, mode,
                       e2t_d, e1t_d, cls1_d, w1_d, b1_d, w2_d, b2_d, ones_d,
                       out_d)

    nc.compile()
    return nc


def _emit_body(nc, tc, bounds, mode, e2t_d, e1t_d, cls1_d, w1_d, b1_d, w2_d,
               b2_d, ones_d, out_d):
    with ExitStack() as ctx:
        consts = ctx.enter_context(tc.tile_pool(name="consts", bufs=1))
        w1 = consts.tile([3, DH], F32)
        b1 = consts.tile([DH, 1], F32)
        w2 = consts.tile([DH, DF], F32)
        b2 = consts.tile([DF, 1], F32)
        ones = consts.tile([DF, DF], F32)
        cls1 = consts.tile([DF, MLOC], BF16)
        nc.sync.dma_start(w1[:], w1_d)
        nc.sync.dma_start(b1[:], b1_d)
        nc.sync.dma_start(w2[:], w2_d)
        nc.sync.dma_start(b2[:], b2_d)
        nc.sync.dma_start(ones[:], ones_d)
        nc.sync.dma_start(cls1[:], cls1_d)

        # persistent main-loop lhsT operand (gated edges1-side features)
        persist = ctx.enter_context(tc.tile_pool(name="persist", bufs=1))
        if mode == "split":
            v1m = persist.tile([3 * DF, NCLS, MLOC], BF16)  # [h1;l1;h1] gated
        elif mode == "f32r":
            v1m = persist.tile([DF, NCLS, MLOC], F32R)
        else:
            v1m = persist.tile([DF, NCLS, MLOC], F32)

        # side-1 pools stay open for the whole body (emission is interleaved
        # into the chunk loop below to avoid serializing the static per-engine
        # schedule on side-1's long dependency chain)
        scr1 = ctx.enter_context(tc.tile_pool(name="scr1", bufs=1))
        s1g = ctx.enter_context(tc.tile_pool(name="s1g", bufs=3))

        def side1_gen():
            """Yield after each instruction; computes v1m from e1t."""
            e1t = scr1.tile([3, MLOC], F32, tag="s1A")
            nc.sync.dma_start(e1t[:], e1t_d)
            yield
            hps1 = ppsum2.tile([DH, 2, CH], F32, tag="pps")
            for c0 in range(0, MLOC, CH):
                nc.tensor.matmul(hps1[:, c0 // CH, :], w1[:],
                                 e1t[:, c0:c0 + CH], start=True, stop=True)
            yield
            h1 = scr1.tile([DH, MLOC], F32, tag="s1B")
            nc.scalar.activation(h1[:], hps1[:].rearrange("p a b -> p (a b)"),
                                 AF.Relu, bias=b1[:], scale=1.0)
            yield
            fps1 = ppsum2.tile([DF, 2, CH], F32, tag="pps")
            for c0 in range(0, MLOC, CH):
                nc.tensor.matmul(fps1[:, c0 // CH, :], w2[:],
                                 h1[:, c0:c0 + CH], start=True, stop=True)
            yield
            sq1 = scr1.tile([DF, 2, CH], F32, tag="s1SQ")
            nc.scalar.activation(sq1[:], fps1[:], AF.Square, bias=b2[:],
                                 scale=1.0)
            yield
            nps1 = ppsum2.tile([DF, 2, CH], F32, tag="pps")
            for j in range(2):
                nc.tensor.matmul(nps1[:, j, :], ones[:], sq1[:, j, :],
                                 start=True, stop=True)
            yield
            nsq = scr1.tile([DF, MLOC], F32, tag="s1N")
            nc.scalar.sqrt(nsq[:], nps1[:].rearrange("p a b -> p (a b)"))
            yield
            nc.vector.reciprocal(nsq[:], nsq[:])
            yield
            u1 = scr1.tile([DF, MLOC], F32, tag="s1U")
            nc.vector.scalar_tensor_tensor(
                u1[:], fps1[:].rearrange("p a b -> p (a b)"), b2[:], nsq[:],
                ALU.add, ALU.mult)
            yield
            if mode == "split":
                hb1 = scr1.tile([DF, MLOC], BF16, tag="s1D")
                nc.scalar.copy(hb1[:], u1[:])
                yield
                rsd1 = scr1.tile([DF, MLOC], F32, tag="s1R")
                nc.vector.tensor_tensor(rsd1[:], u1[:], hb1[:], ALU.subtract)
                yield
                lb1 = scr1.tile([DF, MLOC], BF16, tag="s1E")
                nc.vector.tensor_copy(lb1[:], rsd1[:])
                yield
                for c in range(NCLS):
                    ghc = s1g.tile([DF, MLOC], BF16, tag="s1GH")
                    nc.vector.scalar_tensor_tensor(
                        ghc[:], cls1[:], float(c), hb1[:],
                        ALU.is_equal, ALU.mult)
                    nc.sync.dma_start(v1m[0:DF, c], ghc[:])
                    nc.sync.dma_start(v1m[2 * DF:3 * DF, c], ghc[:])
                    yield
                    glc = s1g.tile([DF, MLOC], BF16, tag="s1GL")
                    nc.vector.scalar_tensor_tensor(
                        glc[:], cls1[:], float(c), lb1[:],
                        ALU.is_equal, ALU.mult)
                    nc.sync.dma_start(v1m[DF:2 * DF, c], glc[:])
                    yield
            else:
                if mode == "f32":
                    v1g = v1m
                else:
                    v1g = scr1.tile([DF, NCLS, MLOC], F32, tag="s1G")
                for c in range(NCLS):
                    nc.vector.scalar_tensor_tensor(
                        v1g[:, c, :], cls1[:], float(c), u1[:],
                        ALU.is_equal, ALU.mult)
                    yield
                if mode == "f32r":
                    nc.vector.tensor_copy(v1m[:], v1g[:])

        # ---- pipelined side-2 + main loop, one 1024-col chunk at a time ----
        # (prologue fp32 matmuls sub-chunk at 512 = fp32 moving-max; all
        # elementwise/copy/DMA ops run at 1024 free for half the instruction
        # overheads and 4KB-contiguous output rows)
        CHO = 2 * CH
        e2pool = ctx.enter_context(tc.tile_pool(name="e2p", bufs=1))
        e2t = e2pool.tile([3, N2], F32)
        nc.sync.dma_start(e2t[:], e2t_d)

        cpool = ctx.enter_context(tc.tile_pool(name="cscr", bufs=2))
        v2pool = ctx.enter_context(tc.tile_pool(name="v2p", bufs=6))
        ppsum2 = ctx.enter_context(tc.tile_pool(name="ppsum2", bufs=2, space="PSUM"))
        mpsum = ctx.enter_context(tc.tile_pool(name="mpsum", bufs=2, space="PSUM"))
        opool = ctx.enter_context(tc.tile_pool(name="osb", bufs=6))
        n_mt = MLOC // 128
        n_chunks = N2 // CHO

        s1 = side1_gen()
        s1_done = False

        def s1_steps(k):
            nonlocal s1_done
            for _ in range(k):
                if next(s1, "end") == "end":
                    s1_done = True
                    return

        def emit_pro_a(chi):
            """MLP + squared-norm matmuls for 1024-col chunk chi."""
            lo = chi * CHO
            hps = ppsum2.tile([DH, 2, CH], F32, tag="pps")
            for j in range(2):
                nc.tensor.matmul(hps[:, j, :], w1[:],
                                 e2t[:, lo + j * CH:lo + (j + 1) * CH],
                                 start=True, stop=True)
            h = cpool.tile([DH, 2, CH], F32, tag="h")
            nc.scalar.activation(h[:], hps[:], AF.Relu, bias=b1[:], scale=1.0)
            fps = ppsum2.tile([DF, 2, CH], F32, tag="pps")
            for j in range(2):
                nc.tensor.matmul(fps[:, j, :], w2[:], h[:, j, :],
                                 start=True, stop=True)
            # f^2 = Square(fps + b2) straight from PSUM (f itself is never
            # materialized; u below re-reads fps)
            sq = cpool.tile([DF, 2, CH], F32, tag="sq")
            nc.scalar.activation(sq[:], fps[:], AF.Square, bias=b2[:], scale=1.0)
            nps = ppsum2.tile([DF, 2, CH], F32, tag="pps")
            for j in range(2):
                nc.tensor.matmul(nps[:, j, :], ones[:], sq[:, j, :],
                                 start=True, stop=True)
            rn = cpool.tile([DF, CHO], F32, tag="rn")
            nc.scalar.sqrt(rn[:], nps[:].rearrange("p a b -> p (a b)"))
            nc.vector.reciprocal(rn[:], rn[:])
            return fps, rn

        def emit_pro_b(chi, frn):
            """normalize + (hi/lo split) -> v2 for chunk chi."""
            fps, rn = frn
            u = cpool.tile([DF, CHO], F32, tag="u")
            # u = (fps + b2) * rn  -- bias-add and normalize fused, from PSUM
            nc.vector.scalar_tensor_tensor(
                u[:], fps[:].rearrange("p a b -> p (a b)"), b2[:], rn[:],
                ALU.add, ALU.mult)
            if mode == "split":
                v2 = v2pool.tile([3 * DF, CHO], BF16, tag="v2")
                # hi part straight into section 0 (lane-aligned with u)
                nc.scalar.copy(v2[0:DF, :], u[:])
                # residual: mixed-dtype subtract reads the bf16 hi back
                rsd = cpool.tile([DF, CHO], F32, tag="rsd")
                nc.vector.tensor_tensor(rsd[:], u[:], v2[0:DF, :], ALU.subtract)
                lb = cpool.tile([DF, CHO], BF16, tag="lb")
                nc.vector.tensor_copy(lb[:], rsd[:])
                # duplicate hi into section 1, lo into section 2 (partition
                # moves need DMA)
                nc.sync.dma_start(v2[DF:2 * DF, :], v2[0:DF, :])
                nc.sync.dma_start(v2[2 * DF:3 * DF, :], lb[:])
            elif mode == "f32r":
                v2 = v2pool.tile([DF, CHO], F32R, tag="v2")
                nc.vector.tensor_copy(v2[:], u[:])
            else:
                v2 = v2pool.tile([DF, CHO], F32, tag="v2")
                nc.vector.tensor_copy(v2[:], u[:])
            return v2

        def emit_main(chi, v2):
            lo, hi = chi * CHO, (chi + 1) * CHO
            pieces = []
            for c in range(NCLS):
                a, b = max(lo, bounds[c]), min(hi, bounds[c + 1])
                if a < b:
                    pieces.append((c, a, b))
            for m in range(n_mt):
                ps = mpsum.tile([128, CHO], F32)
                for (c, a, b) in pieces:
                    # split on the absolute 512-col grid: each matmul must
                    # stay inside one psum bank (and under the ISA
                    # moving-elements limit)
                    a2 = a
                    while a2 < b:
                        b2 = min(b, (a2 - lo) // CH * CH + lo + CH)
                        nc.tensor.matmul(
                            ps[:, a2 - lo:b2 - lo],
                            v1m[:, c, m * 128:(m + 1) * 128],
                            v2[:, a2 - lo:b2 - lo],
                            start=True, stop=True)
                        a2 = b2
                ob = opool.tile([128, CHO], F32)
                if (chi + m) % 2 == 0:
                    nc.scalar.copy(ob[:], ps[:])
                else:
                    nc.vector.tensor_copy(ob[:], ps[:])
                nc.sync.dma_start(out_d[m * 128:(m + 1) * 128, lo:hi], ob[:])

        # interleaved emission: side-1 steps ride along the first chunks'
        # prologues; mains lag the prologue stream by LAG chunks so prologue
        # chain latency stays off the critical path.
        LAG = 2
        v2s = {}
        nxt = 0
        for chi in range(n_chunks):
            if not s1_done:
                s1_steps(10)
            v2s[chi] = emit_pro_b(chi, emit_pro_a(chi))
            if chi + 1 >= LAG and s1_done and nxt <= chi - LAG + 1:
                emit_main(nxt, v2s.pop(nxt))
                nxt += 1
        if not s1_done:
            s1_steps(1000)
        while nxt < n_chunks:
            emit_main(nxt, v2s.pop(nxt))
            nxt += 1


def kernel(**inputs) -> np.ndarray:
    edges1 = np.ascontiguousarray(np.asarray(inputs["edges1"], dtype=np.float32))
    edges2 = np.ascontiguousarray(np.asarray(inputs["edges2"], dtype=np.float32))
    W1 = np.asarray(inputs["W1"], dtype=np.float32)
    b1 = np.asarray(inputs["b1"], dtype=np.float32)
    W2 = np.asarray(inputs["W2"], dtype=np.float32)
    b2 = np.asarray(inputs["b2"], dtype=np.float32)

    cls2 = edges2[:, 3].astype(np.int64)
    order = np.argsort(cls2, kind="stable")
    counts = tuple(int(x) for x in np.bincount(cls2, minlength=NCLS))

    key = (counts, MODE)
    if key not in _cache:
        _cache[key] = _build_program(counts, MODE)
    nc = _cache[key]

    e2s = edges2[order]
    e2t = np.ascontiguousarray(e2s[:, :3].T)  # [3, N2]
    shared = {
        "e2t": e2t,
        "w1": W1,
        "b1": np.ascontiguousarray(b1[:, None]),
        "w2": W2,
        "b2": np.ascontiguousarray(b2[:, None]),
        "ones": np.ones((DF, DF), dtype=np.float32),
    }
    in_maps = []
    for k in range(NCORES):
        sl = slice(k * MLOC, (k + 1) * MLOC)
        e1t = np.ascontiguousarray(edges1[sl, :3].T)  # [3, MLOC]
        c1 = np.ascontiguousarray(
            np.broadcast_to(edges1[sl, 3][None, :], (DF, MLOC))
        ).astype(ml_dtypes.bfloat16)
        in_maps.append({**shared, "e1t": e1t, "cls1": c1})

    res = run_bass_kernel_spmd(nc, in_maps, core_ids=list(range(NCORES)))
    out_sorted = np.concatenate(
        [res.results[k]["out"] for k in range(NCORES)], axis=0)
    out = np.empty((N1, N2), dtype=np.float32)
    out[:, order] = out_sorted
    return out



# revision 63
# speedup vs baseline: 7.0581x; 7.0581x over previous
"""TRN2 Bass kernel for nn_EdgeMLP: masked pairwise cosine similarity.

out[i, j] = [cls1_i == cls2_j] * cos(f(e1_i), f(e2_j)),  f = 2-layer MLP.

Strategy (8 cores, block-diagonal over the class mask):
  - The mask [cls1_i == cls2_j] with 8 classes means only ~1/8 of the
    8192x8192 output is nonzero.  Sort edges1 rows AND edges2 columns by
    class (host-side, pure data movement): the nonzero support becomes 8
    dense class blocks [m_c, n_c].  Core c computes block c = the full
    cosine matrix between class-c rows and class-c columns -- no masking
    logic on device at all.  The host scatters the 8 blocks into the
    zero-initialized full output (the gather/unshard step).
  - All matmuls are f32r (tf32-like, 1 cyc/row): MLP layers, column
    squared-norms (ones-matmul), and the main [32]x[128,N] dot products.
  - Side-1 (rows) norms are computed PARTITION-major via tiny per-m-tile
    matmuls against a ones column, so the 1/|f1_i| row scaling rides the
    PSUM->SBUF output copy for free (per-partition `scale`/`tensor_scalar`
    operand of the copy).  Side-2 norms use the free-major ones-matmul +
    sqrt; the division happens inside the fused (f2+b2)/|f2| DVE op.
  - Output tiles are bf16 (rel err ~2e-3 << 2e-2 gate), halving out-DMA
    bytes; the host upcasts to f32 during the scatter.
  - PSUM is managed as 8 rotating 1-bank tiles; every matmul output fits
    one bank.  Prologue is pipelined in 512-column chunks; the main loop
    leads with 3 column-0 tiles (k-major) so the PE never waits on the
    side-2 chain tail.
  - One uniform program (shapes padded to the max class count) serves all
    8 cores -> single compile, single SPMD dispatch.
"""

import math
import sys

for _p in ("/opt/trn_rl_repo", "/opt/pypackages"):
    if _p not in sys.path:
        sys.path.append(_p)

from contextlib import ExitStack

import numpy as np

import concourse.bass as bass
import concourse.tile as tile
from concourse import bacc, mybir
from concourse.bass_utils import run_bass_kernel_spmd

F32 = mybir.dt.float32
F32R = mybir.dt.float32r
BF16 = mybir.dt.bfloat16
AF = mybir.ActivationFunctionType
ALU = mybir.AluOpType

N1, N2 = 8192, 8192
NCORES = 8
DH, DF, NCLS = 64, 32, 8
BANK = 512  # psum bank, in f32 elements

USE_DIVIDE = False  # stt(add, divide) fails the DVE ISA check on TRN2

_cache: dict = {}
_last_nc = None


def _chunks(n, step=BANK):
    return [(c, min(c + step, n)) for c in range(0, n, step)]


def _pipe_chunks(n):
    """Pipeline chunks: two 256-col warmup chunks (fast first-output), then
    512s, with any sub-512 remainder last.  Every chunk fits one psum bank
    and never crosses the 512 grid."""
    out = [(0, 256), (256, 512)]
    c = 512
    while c < n:
        out.append((c, min(c + 512, n)))
        c += 512
    return out


def _build_program(m_pad: int, n_pad: int, m_top: int):
    nc = bacc.Bacc("TRN2", target_bir_lowering=False, debug=False)

    e12_d = nc.dram_tensor("e12", [3, DH + m_pad + n_pad], F32R,
                           kind="ExternalInput").ap()
    w2_d = nc.dram_tensor("w2r", [DH, DF], F32R, kind="ExternalInput").ap()
    wb_d = nc.dram_tensor("wb", [DH, 2], F32, kind="ExternalInput").ap()
    out_d = nc.dram_tensor("out", [m_pad, n_pad], BF16,
                           kind="ExternalOutput").ap()

    n_mt = m_pad // 128
    ch1 = _chunks(m_pad)   # prologue pipeline chunks
    ch2 = _chunks(n_pad)
    mch = _chunks(n_pad)   # main-loop psum chunks (512 grid)

    with tile.TileContext(nc) as tc, ExitStack() as ctx:
        consts = ctx.enter_context(tc.tile_pool(name="consts", bufs=1))
        work = ctx.enter_context(tc.tile_pool(name="work", bufs=1))
        obuf = ctx.enter_context(tc.tile_pool(name="obuf", bufs=6))
        psum = ctx.enter_context(tc.tile_pool(name="psum", bufs=7,
                                              space="PSUM"))
        pnorm = ctx.enter_context(tc.tile_pool(name="pnorm", bufs=1,
                                               space="PSUM"))

        def ps_tile():
            return psum.tile([128, BANK], F32, tag="ps", name="ps")

        # --- t=0: inputs + ACT table warmup --------------------------------
        e12 = consts.tile([3, DH + m_pad + n_pad], F32R)
        w2 = consts.tile([DH, DF], F32R)
        wb = consts.tile([DH, 2], F32)
        ones = consts.tile([DF, DF], F32R)
        onesf = consts.tile([DF, DF], F32)
        warm = consts.tile([1, 1], F32)
        nc.sync.dma_start(e12[:], e12_d)
        nc.sync.dma_start(w2[:], w2_d)
        nc.scalar.dma_start(wb[:], wb_d)
        nc.gpsimd.memset(onesf[:], 1.0)
        nc.gpsimd.tensor_copy(ones[:], onesf[:])  # f32r rounding copy
        # a Sqrt as ACT's first op pins the table set that covers every
        # activation used here (sqrt_and_others: Sqrt/Square/Identity/Copy),
        # so the 1.3us table load runs once, hidden under the input DMAs
        nc.scalar.sqrt(warm[:], wb[0:1, 0:1])

        w1 = e12[:, 0:DH]
        e1t = e12[:, DH:DH + m_pad]
        e2t = e12[:, DH + m_pad:]
        b1 = wb[:, 0:1]
        b2 = wb[0:DF, 1:2]


        h1 = work.tile([DH, m_pad], F32R, tag="h1")
        f1 = work.tile([DF, m_pad], F32R, tag="f1")
        sq1 = work.tile([DF, m_pad], F32R, tag="sq1")
        rn1 = work.tile([128, 2 * n_mt], F32, tag="rn1")
        h2 = work.tile([DH, n_pad], F32R, tag="h2")
        rt2 = work.tile([DF, n_pad], F32, tag="rt2")
        u2 = work.tile([DF, n_pad], F32R, tag="u2")

        # --- prologue: both sides pipelined over column chunks ------------
        # side 2 chunk: L1 mm -> relu(DVE) -> L2 mm -> sq2(ACT) -> ns mm
        #   -> sqrt(ACT) -> u2 = (fps+b2)/rt2 (DVE, fused divide)
        # side 1 chunk: L1 mm -> relu(Pool) -> L2 mm -> f1(ACT,+b2); one
        #   chunk later: sq1(DVE) -> tiny partition-major ns1p mms ->
        #   sqrt(ACT) -> recip(DVE), so the PE/DVE FIFOs never head-block
        #   on the side-1 norm chain.
        sq2 = work.tile([DF, n_pad], F32R, tag="sq2")
        # ns1p is long-lived (read until the last rn1 sqrt) -- it must NOT
        # occupy a slot of the rotating pool, or every wrap-around request
        # would stall behind it
        ns1p = pnorm.tile([128, BANK], F32, tag="ns1p", name="ns1p")

        def emit_l1(side, k):
            """L1 matmul + fused bias-relu; relu2 on Pool, relu1 on DVE."""
            ch, et = (ch1, e1t) if side == 1 else (ch2, e2t)
            c0, c1 = ch[k]
            w = c1 - c0
            hp = ps_tile()
            nc.tensor.matmul(hp[0:DH, 0:w], w1,
                             et[:, c0:c1],
                             start=True, stop=True)
            if side == 2:
                nc.scalar.activation(h2[:, c0:c1], hp[0:DH, 0:w], AF.Relu,
                                     bias=b1, scale=1.0)
            else:
                nc.vector.tensor_scalar(h1[:, c0:c1], hp[0:DH, 0:w], b1,
                                        0.0, ALU.add, ALU.max)

        def norm_sq(c0, c1):
            nc.gpsimd.tensor_tensor(sq1[:, c0:c1], f1[:, c0:c1],
                                    f1[:, c0:c1], ALU.mult)

        def norm_fin(c0, c1):
            # fp32r matmuls need even free counts, so each norm matmul
            # writes a 2-wide column pair; rn1 is indexed at stride 2
            lo, hi = (c0 + 127) // 128, c1 // 128
            for m in range(lo, hi):
                nc.tensor.matmul(
                    ns1p[:, 2 * m:2 * m + 2],
                    sq1[:, m * 128:(m + 1) * 128],
                    ones[:, 0:2], start=True, stop=True)
            if hi > lo:
                nc.scalar.sqrt(rn1[:, 2 * lo:2 * hi], ns1p[:, 2 * lo:2 * hi])
                nc.vector.reciprocal(rn1[:, 2 * lo:2 * hi],
                                     rn1[:, 2 * lo:2 * hi])

        # ---- main-loop helpers (needed mid-prologue for early emission) --
        nk = len(mch)
        obs = [None] * n_mt
        pss = {}
        # time-aware copy-engine schedule: Pool finishes its prologue work
        # first, so it front-loads the early copies; ACT frees next, DVE
        # last (u2/sq1 tail)
        big_rot = ["a", "v", "a", "v", "a", "v", "a", "v", "a", "a",
                   "v", "a", "v", "a", "v", "a", "v", "a"]
        small_rot = ["v", "a", "v", "v", "a", "v", "v", "a", "v"]
        nbig = [0]
        nsmall = [0]

        def emit_mm(m, k):
            c0, c1 = mch[k]
            w = c1 - c0
            ps = ps_tile()
            pss[(m, k)] = ps
            nc.tensor.matmul(ps[:, 0:w],
                             f1[:, m * 128:(m + 1) * 128],
                             u2[:, c0:c1],
                             start=True, stop=True)

        def emit_copy(m, k):
            c0, c1 = mch[k]
            w = c1 - c0
            ps = pss.pop((m, k))
            ob = obs[m]
            scale = rn1[:, 2 * m:2 * m + 1]
            if w >= 256:
                r = big_rot[nbig[0] % len(big_rot)]
                nbig[0] += 1
            else:
                r = small_rot[nsmall[0] % len(small_rot)]
                nsmall[0] += 1
            if r == "a":
                nc.scalar.activation(ob[:, c0:c1], ps[:, 0:w], AF.Copy,
                                     bias=0.0, scale=scale)
            else:
                nc.vector.tensor_scalar(ob[:, c0:c1], ps[:, 0:w], scale,
                                        None, ALU.mult)

        # Choreographed emission: every engine's in-order queue receives its
        # ops in expected-readiness order.  L1 matmuls + relus for chunk k+1
        # are pre-emitted; side-2's norm chain leads; f1 (Pool) and the
        # side-1 norm pieces ride the gaps; the first column-0 main
        # matmuls are emitted mid-prologue so the PE reaches them the moment
        # u2(0) lands.
        n_ch = max(len(ch1), len(ch2))
        emit_l1(2, 0)
        emit_l1(1, 0)
        if len(ch2) > 1:
            emit_l1(2, 1)
        if len(ch1) > 1:
            emit_l1(1, 1)
        early_mains = []
        done_norms = set()
        for k in range(n_ch):
            in1 = k < len(ch1)
            in2 = k < len(ch2)
            if k + 2 < len(ch2):
                emit_l1(2, k + 2)
            if k + 2 < len(ch1):
                emit_l1(1, k + 2)
            if k == 1:
                norm_sq(128, ch1[0][1])
            if in2:
                c0, c1 = ch2[k]
                w = c1 - c0
                fp2 = ps_tile()
                nc.tensor.matmul(fp2[0:DF, 0:w], w2,
                                 h2[:, c0:c1],
                                 start=True, stop=True)
                nc.scalar.activation(sq2[:, c0:c1], fp2[0:DF, 0:w],
                                     AF.Square, bias=b2, scale=1.0)
            if in1:
                d0, d1 = ch1[k]
                wd = d1 - d0
                fp1 = ps_tile()
                nc.tensor.matmul(fp1[0:DF, 0:wd], w2,
                                 h1[:, d0:d1],
                                 start=True, stop=True)
                nc.scalar.activation(f1[:, d0:d1], fp1[0:DF, 0:wd],
                                     AF.Identity, bias=b2, scale=1.0)
            if k == 0:
                # m-tile-0 fast path: sq1 slots into DVE's idle window just
                # before u2(0), so rn1[0] lands right after the first mains
                norm_sq(0, 128)

            if in2:
                np_ = ps_tile()
                nc.tensor.matmul(np_[0:DF, 0:w], ones,
                                 sq2[:, c0:c1],
                                 start=True, stop=True)
                nc.scalar.sqrt(rt2[:, c0:c1], np_[0:DF, 0:w])
                if USE_DIVIDE:
                    nc.vector.scalar_tensor_tensor(
                        u2[:, c0:c1], fp2[0:DF, 0:w], b2, rt2[:, c0:c1],
                        ALU.add, ALU.divide)
                else:
                    nc.vector.reciprocal(rt2[:, c0:c1], rt2[:, c0:c1])
                    nc.vector.scalar_tensor_tensor(
                        u2[:, c0:c1], fp2[0:DF, 0:w], b2, rt2[:, c0:c1],
                        ALU.add, ALU.mult)
            if k == 0:
                norm_fin(0, 128)
                done_norms.add(0)
            if k == 1:
                norm_fin(128, ch1[0][1])
                for m in range(2):
                    obs[m] = obuf.tile([128, n_pad], BF16, tag="ob",
                                       name="ob")
                    emit_mm(m, 0)
                early_mains.append(2)
        # side-1 norm chunks 1+ are deferred into the mains stream (their
        # rn1 slices are only read by later m-tiles' copies)

        # --- main loop ----------------------------------------------------
        # steady state: finish tile m, then start tile m+LEAD's column 0.
        LEAD = 5
        n_early = early_mains[0] if early_mains else 0
        for m in range(n_early, min(LEAD, n_mt)):
            obs[m] = obuf.tile([128, n_pad], BF16, tag="ob", name="ob")
            emit_mm(m, 0)
        c0end = mch[0][1]
        for m in range(n_mt):
            emit_copy(m, 0)
            if m < 2:
                # early tiles: ship column-chunk 0 the moment it is copied,
                # so the DMA bus starts draining ~3.5us sooner
                nc.sync.dma_start(out_d[m * 128:(m + 1) * 128, 0:c0end],
                                  obs[m][:, 0:c0end])
            for k in range(1, nk):
                emit_mm(m, k)
                emit_copy(m, k)
            if m < len(ch1) - 1:
                # deferred side-1 norm pieces ride between m-tiles
                norm_sq(ch1[m + 1][0], ch1[m + 1][1])
                norm_fin(ch1[m + 1][0], ch1[m + 1][1])
            if m + LEAD < n_mt:
                obs[m + LEAD] = obuf.tile([128, n_pad], BF16, tag="ob",
                                          name="ob")
                emit_mm(m + LEAD, 0)
            r0 = m * 128
            r1 = min((m + 1) * 128, m_top)
            if r1 > r0:
                if m < 2:
                    nc.sync.dma_start(out_d[r0:r1, c0end:], obs[m][0:r1 - r0,
                                                                  c0end:])
                else:
                    nc.sync.dma_start(out_d[r0:r1, :], obs[m][0:r1 - r0, :])

    nc.compile()
    return nc


def kernel(**inputs) -> np.ndarray:
    global _last_nc
    edges1 = np.ascontiguousarray(np.asarray(inputs["edges1"], dtype=np.float32))
    edges2 = np.ascontiguousarray(np.asarray(inputs["edges2"], dtype=np.float32))
    W1 = np.asarray(inputs["W1"], dtype=np.float32)
    b1 = np.asarray(inputs["b1"], dtype=np.float32)
    W2 = np.asarray(inputs["W2"], dtype=np.float32)
    b2 = np.asarray(inputs["b2"], dtype=np.float32)

    cls1 = edges1[:, 3].astype(np.int64)
    cls2 = edges2[:, 3].astype(np.int64)
    order1 = np.argsort(cls1, kind="stable")
    order2 = np.argsort(cls2, kind="stable")
    cnt1 = np.bincount(cls1, minlength=NCLS)
    cnt2 = np.bincount(cls2, minlength=NCLS)
    b1_ = np.concatenate([[0], np.cumsum(cnt1)]).astype(int)
    b2_ = np.concatenate([[0], np.cumsum(cnt2)]).astype(int)

    m_pad = max(128, math.ceil(cnt1.max() / 128) * 128)
    n_pad = max(8, math.ceil(cnt2.max() / 8) * 8)
    m_top = max(8, math.ceil(cnt1.max() / 8) * 8)  # valid-row DMA bound

    key = (m_pad, n_pad, m_top)
    if key not in _cache:
        _cache[key] = _build_program(m_pad, n_pad, m_top)
    nc = _cache[key]
    _last_nc = nc

    # wb: [64, 2] = b1 | b2
    wb = np.zeros((DH, 2), dtype=np.float32)
    wb[:, 0] = b1
    wb[0:DF, 1] = b2

    in_maps = []
    for c in range(NCORES):
        rows = order1[b1_[c]:b1_[c + 1]]
        cols = order2[b2_[c]:b2_[c + 1]]
        e12 = np.zeros((3, DH + m_pad + n_pad), dtype=np.float32)
        e12[:, 0:DH] = W1
        e12[:, DH:DH + len(rows)] = edges1[rows, :3].T
        e12[:, DH + m_pad:DH + m_pad + len(cols)] = edges2[cols, :3].T
        in_maps.append({"e12": e12, "w2r": W2, "wb": wb})

    res = run_bass_kernel_spmd(nc, in_maps, core_ids=list(range(NCORES)))

    out = np.zeros((N1, N2), dtype=np.float32)
    for c in range(NCORES):
        rows = order1[b1_[c]:b1_[c + 1]]
        cols = order2[b2_[c]:b2_[c + 1]]
        blk = np.asarray(res.results[c]["out"])[:len(rows), :len(cols)]
        out[np.ix_(rows, cols)] = blk.astype(np.float32)
    return out


# revision 68
# speedup vs baseline: 7.0871x; 1.0041x over previous
"""TRN2 Bass kernel for nn_EdgeMLP: masked pairwise cosine similarity.

out[i, j] = [cls1_i == cls2_j] * cos(f(e1_i), f(e2_j)),  f = 2-layer MLP.

Strategy (8 cores, block-diagonal over the class mask):
  - The mask [cls1_i == cls2_j] with 8 classes means only ~1/8 of the
    8192x8192 output is nonzero.  Sort edges1 rows AND edges2 columns by
    class (host-side, pure data movement): the nonzero support becomes 8
    dense class blocks [m_c, n_c].  Core c computes block c = the full
    cosine matrix between class-c rows and class-c columns -- no masking
    logic on device at all.  The host scatters the 8 blocks into the
    zero-initialized full output (the gather/unshard step).
  - All matmuls are f32r (tf32-like, 1 cyc/row): MLP layers, column
    squared-norms (ones-matmul), and the main [32]x[128,N] dot products.
  - Side-1 (rows) norms are computed PARTITION-major via tiny per-m-tile
    matmuls against a ones column-pair, so the 1/|f1_i| row scaling rides
    the PSUM->SBUF output copy for free (per-partition `scale` operand of
    the copy).  Side-2 norms use the free-major ones-matmul + sqrt(ACT) +
    reciprocal(DVE) + fused (f2+b2)*rn2 stt (DVE).
  - Output tiles are bf16 (rel err ~2e-3 << 2e-2 gate), halving out-DMA
    bytes; the host upcasts to f32 during the scatter.
  - PSUM is managed as 7 rotating 1-bank tiles (+1 for the long-lived
    norm accumulator); every matmul output fits one bank.  Prologue is
    pipelined in 512-column chunks; emission is choreographed so each
    engine's in-order queue receives ops in readiness order, with the
    first main matmuls and an m-tile-0 norm fast path emitted
    mid-prologue, and the first two tiles' out-DMAs split so the DMA bus
    starts draining early.
  - One uniform program (shapes padded to the max class count) serves all
    8 cores -> single compile, single SPMD dispatch.

  Hardware/ISA constraints discovered the hard way (walrus verifier):
  - GPSIMD (Pool) cannot access PSUM at all -> Pool only gets SBUF->SBUF
    work (the sq1 squares); psum evacuation is ACT+DVE only.
  - fp32r matmul operands must be PRODUCED as f32r (dram tensors and
    every producer op declare f32r; a plain f32->f32r bitcast fails
    verification), and fp32r matmuls need even free-dim counts.
  - scalar_tensor_tensor with op1=divide fails the DVE ISA check.
  - ACT Rsqrt is banned in bass (accuracy); sqrt+reciprocal instead.
"""

import math
import sys

for _p in ("/opt/trn_rl_repo", "/opt/pypackages"):
    if _p not in sys.path:
        sys.path.append(_p)

from contextlib import ExitStack

import numpy as np

import concourse.bass as bass
import concourse.tile as tile
from concourse import bacc, mybir
from concourse.bass_utils import run_bass_kernel_spmd

F32 = mybir.dt.float32
F32R = mybir.dt.float32r
BF16 = mybir.dt.bfloat16
AF = mybir.ActivationFunctionType
ALU = mybir.AluOpType

N1, N2 = 8192, 8192
NCORES = 8
DH, DF, NCLS = 64, 32, 8
BANK = 512  # psum bank, in f32 elements

USE_DIVIDE = False  # stt(add, divide) fails the DVE ISA check on TRN2

_cache: dict = {}
_last_nc = None


def _chunks(n, step=BANK):
    return [(c, min(c + step, n)) for c in range(0, n, step)]


def _pipe_chunks(n):
    """Pipeline chunks: two 256-col warmup chunks (fast first-output), then
    512s, with any sub-512 remainder last.  Every chunk fits one psum bank
    and never crosses the 512 grid."""
    out = [(0, 256), (256, 512)]
    c = 512
    while c < n:
        out.append((c, min(c + 512, n)))
        c += 512
    return out


def _build_program(m_pad: int, n_pad: int, m_top: int):
    nc = bacc.Bacc("TRN2", target_bir_lowering=False, debug=False)

    e12_d = nc.dram_tensor("e12", [3, DH + m_pad + n_pad], F32R,
                           kind="ExternalInput").ap()
    w2_d = nc.dram_tensor("w2r", [DH, DF], F32R, kind="ExternalInput").ap()
    wb_d = nc.dram_tensor("wb", [DH, 2], F32, kind="ExternalInput").ap()
    out_d = nc.dram_tensor("out", [m_pad, n_pad], BF16,
                           kind="ExternalOutput").ap()

    n_mt = m_pad // 128
    ch1 = _chunks(m_pad)   # prologue pipeline chunks
    ch2 = _chunks(n_pad)
    mch = _chunks(n_pad)   # main-loop psum chunks (512 grid)

    with tile.TileContext(nc) as tc, ExitStack() as ctx:
        consts = ctx.enter_context(tc.tile_pool(name="consts", bufs=1))
        work = ctx.enter_context(tc.tile_pool(name="work", bufs=1))
        obuf = ctx.enter_context(tc.tile_pool(name="obuf", bufs=6))
        psum = ctx.enter_context(tc.tile_pool(name="psum", bufs=7,
                                              space="PSUM"))
        pnorm = ctx.enter_context(tc.tile_pool(name="pnorm", bufs=1,
                                               space="PSUM"))

        def ps_tile():
            return psum.tile([128, BANK], F32, tag="ps", name="ps")

        # --- t=0: inputs + ACT table warmup --------------------------------
        e12 = consts.tile([3, DH + m_pad + n_pad], F32R)
        w2 = consts.tile([DH, DF], F32R)
        wb = consts.tile([DH, 2], F32)
        ones = consts.tile([DF, DF], F32R)
        onesf = consts.tile([DF, DF], F32)
        warm = consts.tile([1, 1], F32)
        nc.sync.dma_start(e12[:], e12_d)
        nc.sync.dma_start(w2[:], w2_d)
        nc.scalar.dma_start(wb[:], wb_d)
        nc.gpsimd.memset(onesf[:], 1.0)
        nc.gpsimd.tensor_copy(ones[:], onesf[:])  # f32r rounding copy
        # a Sqrt as ACT's first op pins the table set that covers every
        # activation used here (sqrt_and_others: Sqrt/Square/Identity/Copy),
        # so the 1.3us table load runs once, hidden under the input DMAs
        nc.scalar.sqrt(warm[:], wb[0:1, 0:1])

        w1 = e12[:, 0:DH]
        e1t = e12[:, DH:DH + m_pad]
        e2t = e12[:, DH + m_pad:]
        b1 = wb[:, 0:1]
        b2 = wb[0:DF, 1:2]


        h1 = work.tile([DH, m_pad], F32R, tag="h1")
        f1 = work.tile([DF, m_pad], F32R, tag="f1")
        sq1 = work.tile([DF, m_pad], F32R, tag="sq1")
        rn1 = work.tile([128, 2 * n_mt], F32, tag="rn1")
        h2 = work.tile([DH, n_pad], F32R, tag="h2")
        rt2 = work.tile([DF, n_pad], F32, tag="rt2")
        u2 = work.tile([DF, n_pad], F32R, tag="u2")

        # --- prologue: both sides pipelined over column chunks ------------
        # side 2 chunk: L1 mm -> relu(DVE) -> L2 mm -> sq2(ACT) -> ns mm
        #   -> sqrt(ACT) -> u2 = (fps+b2)/rt2 (DVE, fused divide)
        # side 1 chunk: L1 mm -> relu(Pool) -> L2 mm -> f1(ACT,+b2); one
        #   chunk later: sq1(DVE) -> tiny partition-major ns1p mms ->
        #   sqrt(ACT) -> recip(DVE), so the PE/DVE FIFOs never head-block
        #   on the side-1 norm chain.
        sq2 = work.tile([DF, n_pad], F32R, tag="sq2")
        # ns1p is long-lived (read until the last rn1 sqrt) -- it must NOT
        # occupy a slot of the rotating pool, or every wrap-around request
        # would stall behind it
        ns1p = pnorm.tile([128, BANK], F32, tag="ns1p", name="ns1p")

        def emit_l1(side, k):
            """L1 matmul + fused bias-relu; relu2 on Pool, relu1 on DVE."""
            ch, et = (ch1, e1t) if side == 1 else (ch2, e2t)
            c0, c1 = ch[k]
            w = c1 - c0
            hp = ps_tile()
            nc.tensor.matmul(hp[0:DH, 0:w], w1,
                             et[:, c0:c1],
                             start=True, stop=True)
            if side == 2:
                nc.scalar.activation(h2[:, c0:c1], hp[0:DH, 0:w], AF.Relu,
                                     bias=b1, scale=1.0)
            else:
                nc.vector.tensor_scalar(h1[:, c0:c1], hp[0:DH, 0:w], b1,
                                        0.0, ALU.add, ALU.max)

        def norm_sq(c0, c1):
            nc.gpsimd.tensor_tensor(sq1[:, c0:c1], f1[:, c0:c1],
                                    f1[:, c0:c1], ALU.mult)

        def norm_fin(c0, c1):
            # fp32r matmuls need even free counts, so each norm matmul
            # writes a 2-wide column pair; rn1 is indexed at stride 2
            lo, hi = (c0 + 127) // 128, c1 // 128
            for m in range(lo, hi):
                nc.tensor.matmul(
                    ns1p[:, 2 * m:2 * m + 2],
                    sq1[:, m * 128:(m + 1) * 128],
                    ones[:, 0:2], start=True, stop=True)
            if hi > lo:
                nc.scalar.sqrt(rn1[:, 2 * lo:2 * hi], ns1p[:, 2 * lo:2 * hi])
                nc.vector.reciprocal(rn1[:, 2 * lo:2 * hi],
                                     rn1[:, 2 * lo:2 * hi])

        # ---- main-loop helpers (needed mid-prologue for early emission) --
        nk = len(mch)
        obs = [None] * n_mt
        pss = {}
        # time-aware copy-engine schedule: Pool finishes its prologue work
        # first, so it front-loads the early copies; ACT frees next, DVE
        # last (u2/sq1 tail)
        big_rot = ["a", "v", "a", "v", "a", "v", "a", "v", "a", "a",
                   "v", "a", "v", "a", "v", "a", "v", "a"]
        small_rot = ["v", "a", "v", "v", "a", "v", "v", "a", "v"]
        nbig = [0]
        nsmall = [0]

        def emit_mm(m, k):
            c0, c1 = mch[k]
            w = c1 - c0
            ps = ps_tile()
            pss[(m, k)] = ps
            nc.tensor.matmul(ps[:, 0:w],
                             f1[:, m * 128:(m + 1) * 128],
                             u2[:, c0:c1],
                             start=True, stop=True)

        def emit_copy(m, k):
            c0, c1 = mch[k]
            w = c1 - c0
            ps = pss.pop((m, k))
            ob = obs[m]
            scale = rn1[:, 2 * m:2 * m + 1]
            if w >= 256:
                r = big_rot[nbig[0] % len(big_rot)]
                nbig[0] += 1
            else:
                r = small_rot[nsmall[0] % len(small_rot)]
                nsmall[0] += 1
            if r == "a":
                nc.scalar.activation(ob[:, c0:c1], ps[:, 0:w], AF.Copy,
                                     bias=0.0, scale=scale)
            else:
                nc.vector.tensor_scalar(ob[:, c0:c1], ps[:, 0:w], scale,
                                        None, ALU.mult)

        # Choreographed emission: every engine's in-order queue receives its
        # ops in expected-readiness order.  L1 matmuls + relus for chunk k+1
        # are pre-emitted; side-2's norm chain leads; f1 (Pool) and the
        # side-1 norm pieces ride the gaps; the first column-0 main
        # matmuls are emitted mid-prologue so the PE reaches them the moment
        # u2(0) lands.
        n_ch = max(len(ch1), len(ch2))
        emit_l1(2, 0)
        emit_l1(1, 0)
        if len(ch2) > 1:
            emit_l1(2, 1)
        if len(ch1) > 1:
            emit_l1(1, 1)
        early_mains = []
        done_norms = set()
        for k in range(n_ch):
            in1 = k < len(ch1)
            in2 = k < len(ch2)
            if k + 2 < len(ch2):
                emit_l1(2, k + 2)
            if k + 2 < len(ch1):
                emit_l1(1, k + 2)
            if k == 1:
                norm_sq(128, ch1[0][1])
            if in2:
                c0, c1 = ch2[k]
                w = c1 - c0
                fp2 = ps_tile()
                nc.tensor.matmul(fp2[0:DF, 0:w], w2,
                                 h2[:, c0:c1],
                                 start=True, stop=True)
                nc.scalar.activation(sq2[:, c0:c1], fp2[0:DF, 0:w],
                                     AF.Square, bias=b2, scale=1.0)
            if in1:
                d0, d1 = ch1[k]
                wd = d1 - d0
                fp1 = ps_tile()
                nc.tensor.matmul(fp1[0:DF, 0:wd], w2,
                                 h1[:, d0:d1],
                                 start=True, stop=True)
                nc.scalar.activation(f1[:, d0:d1], fp1[0:DF, 0:wd],
                                     AF.Identity, bias=b2, scale=1.0)
            if k == 0:
                # m-tile-0 fast path: sq1 slots into DVE's idle window just
                # before u2(0), so rn1[0] lands right after the first mains
                norm_sq(0, 128)

            if in2:
                np_ = ps_tile()
                nc.tensor.matmul(np_[0:DF, 0:w], ones,
                                 sq2[:, c0:c1],
                                 start=True, stop=True)
                nc.scalar.sqrt(rt2[:, c0:c1], np_[0:DF, 0:w])
                if USE_DIVIDE:
                    nc.vector.scalar_tensor_tensor(
                        u2[:, c0:c1], fp2[0:DF, 0:w], b2, rt2[:, c0:c1],
                        ALU.add, ALU.divide)
                else:
                    nc.vector.reciprocal(rt2[:, c0:c1], rt2[:, c0:c1])
                    nc.vector.scalar_tensor_tensor(
                        u2[:, c0:c1], fp2[0:DF, 0:w], b2, rt2[:, c0:c1],
                        ALU.add, ALU.mult)
            if k == 0:
                norm_fin(0, 128)
                done_norms.add(0)
            if k == 1:
                norm_fin(128, ch1[0][1])
                for m in range(2):
                    obs[m] = obuf.tile([128, n_pad], BF16, tag="ob",
                                       name="ob")
                    emit_mm(m, 0)
                early_mains.append(2)
        # side-1 norm chunks 1+ are deferred into the mains stream (their
        # rn1 slices are only read by later m-tiles' copies)

        # --- main loop ----------------------------------------------------
        # steady state: finish tile m, then start tile m+LEAD's column 0.
        LEAD = 5
        n_early = early_mains[0] if early_mains else 0
        for m in range(n_early, min(LEAD, n_mt)):
            obs[m] = obuf.tile([128, n_pad], BF16, tag="ob", name="ob")
            emit_mm(m, 0)
        c0end = mch[0][1]
        for m in range(n_mt):
            emit_copy(m, 0)
            if m < 2:
                # early tiles: ship column-chunk 0 the moment it is copied,
                # so the DMA bus starts draining ~3.5us sooner
                nc.sync.dma_start(out_d[m * 128:(m + 1) * 128, 0:c0end],
                                  obs[m][:, 0:c0end])
            for k in range(1, nk):
                emit_mm(m, k)
                emit_copy(m, k)
            if m < len(ch1) - 1:
                # deferred side-1 norm pieces ride between m-tiles
                norm_sq(ch1[m + 1][0], ch1[m + 1][1])
                norm_fin(ch1[m + 1][0], ch1[m + 1][1])
            if m + LEAD < n_mt:
                obs[m + LEAD] = obuf.tile([128, n_pad], BF16, tag="ob",
                                          name="ob")
                emit_mm(m + LEAD, 0)
            r0 = m * 128
            r1 = min((m + 1) * 128, m_top)
            if r1 > r0:
                if m < 2:
                    nc.sync.dma_start(out_d[r0:r1, c0end:], obs[m][0:r1 - r0,
                                                                  c0end:])
                else:
                    nc.sync.dma_start(out_d[r0:r1, :], obs[m][0:r1 - r0, :])

    nc.compile()
    return nc


def kernel(**inputs) -> np.ndarray:
    global _last_nc
    edges1 = np.ascontiguousarray(np.asarray(inputs["edges1"], dtype=np.float32))
    edges2 = np.ascontiguousarray(np.asarray(inputs["edges2"], dtype=np.float32))
    W1 = np.asarray(inputs["W1"], dtype=np.float32)
    b1 = np.asarray(inputs["b1"], dtype=np.float32)
    W2 = np.asarray(inputs["W2"], dtype=np.float32)
    b2 = np.asarray(inputs["b2"], dtype=np.float32)

    cls1 = edges1[:, 3].astype(np.int64)
    cls2 = edges2[:, 3].astype(np.int64)
    order1 = np.argsort(cls1, kind="stable")
    order2 = np.argsort(cls2, kind="stable")
    cnt1 = np.bincount(cls1, minlength=NCLS)
    cnt2 = np.bincount(cls2, minlength=NCLS)
    b1_ = np.concatenate([[0], np.cumsum(cnt1)]).astype(int)
    b2_ = np.concatenate([[0], np.cumsum(cnt2)]).astype(int)

    m_pad = max(128, math.ceil(cnt1.max() / 128) * 128)
    n_pad = max(8, math.ceil(cnt2.max() / 8) * 8)
    m_top = max(8, math.ceil(cnt1.max() / 8) * 8)  # valid-row DMA bound

    key = (m_pad, n_pad, m_top)
    if key not in _cache:
        _cache[key] = _build_program(m_pad, n_pad, m_top)
    nc = _cache[key]
    _last_nc = nc

    # wb: [64, 2] = b1 | b2
    wb = np.zeros((DH, 2), dtype=np.float32)
    wb[:, 0] = b1
    wb[0:DF, 1] = b2

    in_maps = []
    for c in range(NCORES):
        rows = order1[b1_[c]:b1_[c + 1]]
        cols = order2[b2_[c]:b2_[c + 1]]
        e12 = np.zeros((3, DH + m_pad + n_pad), dtype=np.float32)
        e12[:, 0:DH] = W1
        e12[:, DH:DH + len(rows)] = edges1[rows, :3].T
        e12[:, DH + m_pad:DH + m_pad + len(cols)] = edges2[cols, :3].T
        in_maps.append({"e12": e12, "w2r": W2, "wb": wb})

    res = run_bass_kernel_spmd(nc, in_maps, core_ids=list(range(NCORES)))

    out = np.zeros((N1, N2), dtype=np.float32)
    for c in range(NCORES):
        rows = order1[b1_[c]:b1_[c + 1]]
        cols = order2[b2_[c]:b2_[c + 1]]
        blk = np.asarray(res.results[c]["out"])[:len(rows), :len(cols)]
        out[np.ix_(rows, cols)] = blk.astype(np.float32)
    return out
